# revision 1
# baseline (speedup 1.0000x reference)
"""Trainium2 Bass kernel for nn_BoundaryLoss2 (dice + BCE + boundary loss).

Strategy (data-parallel over batch, one sample per core, 8 cores):
  The expensive part is the exact euclidean distance transform (EDT) of the
  target mask (and its complement) per sample:
      d2[i,j] = min_{di,dj} ( di^2 + dj^2 : mask[i+di, j+dj] )
  decomposed separably into a vertical pass (g = vertical L1 distance) and a
  horizontal parabola pass  w2[i,j] = min_dj ( g[i,j+dj]^2 + dj^2 ).

  Vertical pass runs on the (otherwise idle) tensor engine as a band matmul
      S[i,j] = sum_i' 4^(-|i-i'|) * mask[i',j]
  Since at most two mask pixels exist per distance, S in [4^-g, 8/3*4^-g), so
  g = ceil((127 - exponent(S)) / 2) = (128 - exponent(S)) >> 1   exactly
  (the band weights are exact powers of four, products are exact in bf16 and
  f32 accumulation is monotone, so the leading term always sets the
  exponent).  The exponent is extracted with integer shift ops on the DVE and
  squared on the scalar engine directly into the padded parabola tile.
  Image rows are interleaved two-per-partition ([p, q, j] = img[2p+q, j]) so
  all DMA transfers are contiguous; both masks (t, 1-t) are concatenated in
  the moving operand so each band block needs one N=512 matmul.

  Horizontal pass is a windowed min-plus over shifts |dj| <= K executed as
  tensor_tensor(min) + add(d^2) + tensor_tensor(min) chains on DVE + ACT.
  The windowed result is *exact* iff max(w2) <= K^2, verified on device via a
  fused max-accumulate; a host numpy fallback guarantees correctness
  otherwise (never taken for 50%-density random masks, max true d2 is 5-9).

  d1 (distance to positives) is 0 on positives and d0 (to negatives) is 0 on
  negatives, so the reference's signed map res = d1*(1-t) - (d0-1)*t satisfies
  sig*res = sig*d1 - sig*d0 + sig*t summed per sample.  All loss terms reduce
  to per-partition partial sums -> [128, 8] per-core output, combined on host.
"""

import numpy as np
import ml_dtypes

import concourse.bacc as bacc
import concourse.bass as bass
import concourse.tile as tile
from concourse import mybir
from concourse.bass_utils import run_bass_kernel_spmd

P = 128
H = 256
W = 256
NCORES = 8
B = 8
K = 3  # window radius; result exact iff max(d2) <= K*K (checked on device)
BIG = 30000.0
GAP = 8  # border gap in the parabola tile (>= K, 8 keeps alignment)
SMOOTH = 1e-5
F32 = mybir.dt.float32
BF16 = mybir.dt.bfloat16
U32 = mybir.dt.uint32

# stats column layout
S_SIG, S_T, S_LT, S_ST, S_SP, S_SD1, S_MAXW2, S_SD0 = range(8)


def make_wband():
    """[4,128,128] bf16 band-weight blocks for the interleaved row layout
    (partition p holds image rows 2p and 2p+1): block qs*2+qo maps src plane
    qs to out plane qo: W[k,m] = 4^-|(2m+qo)-(2k+qs)|. Exact powers of 4."""
    k = np.arange(P)
    w = np.zeros((4, P, P), dtype=np.float64)
    for qs in (0, 1):
        for qo in (0, 1):
            dd = np.abs((2 * k[None, :] + qo) - (2 * k[:, None] + qs))
            e = -2.0 * dd.astype(np.float64)
            w[qs * 2 + qo] = np.where(e >= -126, np.exp2(e), 0.0)
    return w.astype(ml_dtypes.bfloat16)


def build_boundary_loss_core(tc, stats_out, logits_in, targets_in, wband_in,
                             use_softplus=False):
    """Emit the per-core kernel. DRAM APs: stats_out [P,8] f32,
    logits_in/targets_in [H,W] f32, wband_in [3,P,P] bf16."""
    nc = tc.nc
    Alu = mybir.AluOpType
    Act = mybir.ActivationFunctionType
    WP = W + 2 * GAP  # padded parabola row width

    with (
        tc.tile_pool(name="consts", bufs=1) as consts,
        tc.tile_pool(name="work", bufs=1) as work,
        tc.tile_pool(name="psum", bufs=4, space=bass.MemorySpace.PSUM) as psum,
    ):
        # ---- load inputs, interleaved rows [p, q, j] = img[2p+q, j] ----
        t_src = targets_in.rearrange("(p q) w -> p q w", q=2)
        l_src = logits_in.rearrange("(p q) w -> p q w", q=2)
        mcat = work.tile([P, 2, 2, W], BF16)  # [p, m, qs, j]
        t_bf = mcat[:, 0]
        nt_bf = mcat[:, 1]
        wb = consts.tile([P, 4, P], BF16)
        l_b = work.tile([P, 2, W], F32)
        # queue plan: each engine issues in matmul-need order
        nc.sync.dma_start(out=t_bf[0:64], in_=t_src[0:64])
        nc.scalar.dma_start(out=t_bf[64:P], in_=t_src[64:P])
        nc.gpsimd.dma_start(out=wb[:, 0], in_=wband_in[0])
        nc.sync.dma_start(out=wb[:, 1], in_=wband_in[1])
        nc.scalar.dma_start(out=wb[:, 2], in_=wband_in[2])
        nc.gpsimd.dma_start(out=wb[:, 3], in_=wband_in[3])
        nc.gpsimd.dma_start(out=l_b[0:64], in_=l_src[0:64])
        nc.sync.dma_start(out=l_b[64:P], in_=l_src[64:P])

        bias0 = consts.tile([P, 1], F32)
        nc.gpsimd.memset(bias0, 0.0)
        bias1 = consts.tile([P, 1], F32)
        nc.gpsimd.memset(bias1, 1.0)

        bias_d2 = {}
        for d in (2,):
            bt = consts.tile([P, 1], F32, name=f"bias_d2_{d}")
            nc.gpsimd.memset(bt, float(d * d))
            bias_d2[d] = bt

        # ---- negatives mask: 1 - t ----
        nc.vector.tensor_scalar(nt_bf, t_bf, -1.0, 1.0, op0=Alu.mult, op1=Alu.add)

        # ---- vertical pass: band matmul + exponent extraction ----
        g2b = work.tile([P, 2, 2, WP], BF16)  # [p, m, g, GAP+j]
        nc.gpsimd.memset(g2b[:, :, :, 0:GAP], BIG)
        nc.gpsimd.memset(g2b[:, :, :, GAP + W:], BIG)
        s_ps = psum.tile([P, 2, 2, W], F32)  # [p, qo, m, j]
        for qs in (0, 1):
            for qo in (0, 1):
                nc.tensor.matmul(
                    s_ps[:, qo], wb[:, qs * 2 + qo], mcat[:, :, qs, :],
                    start=(qs == 0), stop=(qs == 1))
        e32 = work.tile([P, 2, 2, W], U32)  # [p, m, qo, j]
        me = work.tile([P, 2, 2, W], U32)
        dd = work.tile([P, 2, 2, W], U32)
        for qo in (0, 1):
            nc.vector.tensor_scalar(
                e32[:, :, qo], s_ps.bitcast(U32)[:, qo].rearrange(
                    "p m j -> p m j"),
                23, None, op0=Alu.logical_shift_right)
            nc.vector.tensor_scalar(
                me[:, :, qo], e32[:, :, qo], -1.0, 128.0,
                op0=Alu.mult, op1=Alu.add)
            dd_call = nc.vector.tensor_scalar(
                dd[:, :, qo], me[:, :, qo], 1, None,
                op0=Alu.logical_shift_right)
            nc.scalar.activation(
                g2b[:, :, qo, GAP:GAP + W], dd[:, :, qo], Act.Square,
                bias=bias0)

        # ---- windowed parabola pass along columns ----
        def sh(d):
            return g2b[:, :, :, GAP + d:GAP + d + W]

        acc = work.tile([P, 2, 2, W], BF16)
        for d in range(1, K + 1):
            u = work.tile([P, 2, 2, W], BF16, name=f"u{d}")
            nc.vector.tensor_tensor(u, sh(-d), sh(d), Alu.min)
            if d in bias_d2:
                nc.scalar.activation(u, u, Act.Identity, bias=bias_d2[d])
            else:
                nc.vector.tensor_scalar(u, u, float(d * d), None, op0=Alu.add)
            if d == 1:
                nc.vector.tensor_tensor(acc, sh(0), u, Alu.min)
            else:
                nc.vector.tensor_tensor(acc, acc, u, Alu.min)

        w2 = acc  # [p, m, q, j]

        stats = work.tile([P, 8], F32)
        nc.vector.memset(stats, 0.0)

        # ---- distances and loss terms ----
        dst = work.tile([P, 2, 2, W], F32)  # [p, m, q, j]; m=0 -> d1, m=1 -> d0
        nc.scalar.activation(dst[:, 0], w2[:, 0], Act.Sqrt, bias=bias0)
        nc.scalar.activation(dst[:, 1], w2[:, 1], Act.Sqrt, bias=bias0)
        chk = work.tile([P, 2, 2, W], BF16)
        nc.vector.tensor_scalar(
            chk, w2, float(K * K), 0.0, op0=Alu.subtract, op1=Alu.max,
            accum_out=stats[:, S_MAXW2:S_MAXW2 + 1])

        sig = work.tile([P, 2, W], F32)
        nc.scalar.activation(
            sig, l_b, Act.Sigmoid, bias=bias0, accum_out=stats[:, S_SIG:S_SIG + 1])
        sp = work.tile([P, 2, W], F32)
        if use_softplus:
            nc.scalar.activation(
                sp, l_b, Act.Softplus, bias=bias0,
                accum_out=stats[:, S_SP:S_SP + 1])
        else:  # softplus = ln(1 + exp(l))
            ex = work.tile([P, 2, W], F32)
            nc.scalar.activation(ex, l_b, Act.Exp, bias=bias0)
            nc.scalar.activation(
                sp, ex, Act.Ln, bias=bias1, accum_out=stats[:, S_SP:S_SP + 1])

        lt = work.tile([P, 2, W], F32)
        nc.vector.scalar_tensor_tensor(
            lt, l_b, 1.0, t_bf, op0=Alu.mult, op1=Alu.mult,
            accum_out=stats[:, S_LT:S_LT + 1])
        st = work.tile([P, 2, W], F32)
        st_call = nc.vector.scalar_tensor_tensor(
            st, sig, 1.0, t_bf, op0=Alu.mult, op1=Alu.mult,
            accum_out=stats[:, S_ST:S_ST + 1])
        from concourse.tile_rust import add_dep_helper
        add_dep_helper(st_call.ins, dd_call.ins, sync=False,
                       reason="keep DVE free for the EDT chain")
        sd1 = work.tile([P, 2, W], F32)
        nc.vector.scalar_tensor_tensor(
            sd1, sig, 1.0, dst[:, 0], op0=Alu.mult, op1=Alu.mult,
            accum_out=stats[:, S_SD1:S_SD1 + 1])
        sd0 = work.tile([P, 2, W], F32)
        nc.vector.scalar_tensor_tensor(
            sd0, sig, 1.0, dst[:, 1], op0=Alu.mult, op1=Alu.mult,
            accum_out=stats[:, S_SD0:S_SD0 + 1])

        nc.sync.dma_start(out=stats_out, in_=stats)


_CACHE = {}


def _patch_act_tables():
    """Make exp and ln resolve to the combined natural_log_exp table (one
    ACT_TABLE_LOAD instead of two): empty out the single-function sets the
    greedy table chooser would otherwise pick first."""
    from concourse import hw_specs
    if getattr(bacc, "_act_tables_patched", False):
        return
    orig = bacc.get_activation_tables

    keep = ("sigmoid_and_others", "sqrt_and_others",
            "natural_log_exp_and_others")
    Act = mybir.ActivationFunctionType
    needed = {Act.Sigmoid, Act.Sqrt, Act.Exp, Act.Ln, Act.Square,
              Act.Copy, Act.Identity, Act.Relu}

    def patched(arch):
        tabs = orig(arch)
        covered = set()
        for name in keep:
            covered |= tabs.get(name, set())
        if not needed.issubset(covered):
            return tabs  # unknown act_info layout: leave untouched
        for name in tabs:
            if name not in keep:
                tabs[name] = set()
        return tabs

    bacc.get_activation_tables = patched
    bacc._act_tables_patched = True


def _get_nc():
    if "nc" not in _CACHE:
        _patch_act_tables()
        nc = bacc.Bacc("TRN2", target_bir_lowering=False, debug=False)
        logits_in = nc.dram_tensor("logits", (H, W), F32, kind="ExternalInput").ap()
        targets_in = nc.dram_tensor(
            "targets16", (H, W), BF16, kind="ExternalInput").ap()
        wband_in = nc.dram_tensor("wband", (4, P, P), BF16, kind="ExternalInput").ap()
        stats_out = nc.dram_tensor("stats", (P, 8), F32, kind="ExternalOutput").ap()
        with tile.TileContext(nc) as tc:
            build_boundary_loss_core(tc, stats_out, logits_in, targets_in, wband_in)
        nc.compile()
        _CACHE["nc"] = nc
    return _CACHE["nc"]


def combine_stats(stats, t_sums):
    """stats: (NCORES, P, 8), t_sums: (NCORES,) host sums of targets ->
    scalar loss (np.float32). None if the windowed EDT was not provably
    exact (caller must fall back)."""
    if float(stats[:, :, S_MAXW2].sum()) != 0.0:
        return None
    s = stats.sum(axis=1, dtype=np.float64)  # (NCORES, 8)
    n = float(B * H * W)
    s_sig, s_t = s[:, S_SIG], t_sums
    s_lt, s_st = s[:, S_LT], s[:, S_ST]
    s_sp = s[:, S_SP]
    s_sdq = s[:, S_SD1] - s[:, S_SD0]
    has_pos = s_t > 0
    inter = s_st.sum()
    union = s_sig.sum() + s_t.sum() + SMOOTH
    dice = 1.0 - (2.0 * inter + SMOOTH) / union
    bce = (s_sp.sum() - s_lt.sum()) / n
    bdy = np.where(has_pos, s_sdq + s_st, 0.0).sum() / n
    return np.float32(0.5 * dice + 0.5 * bce + 0.5 * bdy)


def run_device(logits, targets, trace=False, trace_cores=None):
    l = np.ascontiguousarray(np.asarray(logits, np.float32).reshape(NCORES, H, W))
    t = np.ascontiguousarray(np.asarray(targets, np.float32).reshape(NCORES, H, W))
    wband = make_wband()
    t16 = t.astype(ml_dtypes.bfloat16)
    in_maps = [
        {"logits": l[i], "targets16": t16[i], "wband": wband}
        for i in range(NCORES)
    ]
    nc = _get_nc()
    res = run_bass_kernel_spmd(
        nc, in_maps, core_ids=list(range(NCORES)), trace=trace,
        trace_cores=trace_cores)
    stats = np.stack([res.results[i]["stats"] for i in range(NCORES)])
    return stats, res


# ---------------- host fallback (exact reference semantics) ----------------

def _edt_np(mask):
    """Exact EDT (distance to nearest True) matching the reference."""
    h, w = mask.shape
    big = float(h * w)
    c = np.where(mask, 0.0, np.inf)
    f = np.empty((h, w))
    s = np.full((w,), big)
    for i in range(h):
        s = np.minimum(s + 1.0, c[i])
        f[i] = s
    g = np.empty((h, w))
    s = np.full((w,), big)
    for i in reversed(range(h)):
        s = np.minimum(s + 1.0, f[i])
        g[i] = s
    g2 = g * g
    jj = np.arange(w, dtype=np.float64)
    dj2 = (jj[:, None] - jj[None, :]) ** 2  # (j_out, j_src)
    d2 = np.empty((h, w))
    for i in range(h):
        d2[i] = (g2[i][None, :] + dj2).min(axis=1)
    return np.sqrt(d2)


def _fallback_loss(logits, targets):
    l = np.asarray(logits, np.float64).reshape(B, H, W)
    t = np.asarray(targets, np.float64).reshape(B, H, W)
    sig = 1.0 / (1.0 + np.exp(-l))
    inter = (sig * t).sum()
    union = sig.sum() + t.sum() + SMOOTH
    dice = 1.0 - (2.0 * inter + SMOOTH) / union
    bce = (np.logaddexp(l, 0.0) - l * t).mean()
    bdy_sum = 0.0
    for b_i in range(B):
        m = t[b_i] > 0.5
        if not m.any():
            continue
        d1 = _edt_np(m)
        d0 = _edt_np(~m)
        res = d1 * (1.0 - t[b_i]) - (d0 - 1.0) * t[b_i]
        bdy_sum += (sig[b_i] * res).sum()
    bdy = bdy_sum / float(B * H * W)
    return np.float32(0.5 * dice + 0.5 * bce + 0.5 * bdy)


def kernel(logits, targets):
    stats, _ = run_device(logits, targets)
    t_sums = np.asarray(targets, np.float64).reshape(NCORES, -1).sum(axis=1)
    loss = combine_stats(stats, t_sums)
    if loss is None:
        loss = _fallback_loss(logits, targets)
    return np.array(loss, dtype=np.float32)

